# revision 7
# baseline (speedup 1.0000x reference)
"""Trainium2 Bass kernel for nn_MultiHeadAttention_45672682226228.

The reference module computes multi-head attention but everything except the
V projection is dead code (DCE'd under jit): the returned value is

    out[b, s, 64*h + q] = x[b, s, 768 + 64*h + q]
                        + sum_d x[b, s, 256*h + d] * W_v[q, d]

i.e. a per-token block-diagonal matmul (4 heads x [256 -> 64]) plus a
residual add of the last head's input slice.  W_q / W_k are unused.

Kernel strategy (v4):
  * Data-parallel over batch B=16 -> 2 batches (8192 tokens) per core.
  * x is pre-transposed and cast to bf16 on the HOST, so the device streams
    xT [1024, 8192] = 8 chunks of [128, 8192] straight into accumulating PE
    matmuls - no on-chip transposes (bf16 error ~3e-3, gate is 2e-2).
  * All 4 heads share W_v, so the only weights are A = W_v.T[0:128] and
    B = W_v.T[128:256], both [128, 64].  M=64 means two matmuls are packed
    side-by-side in the PE array via column tiling (tile_position (0,0) /
    (0,64)), halving PE streaming time:
      outT[  0:128] (heads 0,1): (A@x0 || A@x2), (B@x1 || B@x3)
      outT[128:256] (heads 2,3): (A@x4 || A@x6), (B@x5 || B@x7),
                                 (D0@x7 || D1@x7)   <- residual cols 128:255
    The cc0 residual (xT rows 768:896 = chunk 6, partition-aligned with
    output cols 0:128) is added by the DVE during PSUM evacuation; the cc1
    residual rides two diagonal weight blocks so evacuation can use the
    Scalar engine (which cannot do two-tensor adds).
  * Matmuls are emitted tile-major: each input tile is consumed for all 4
    groups the moment it lands, so the last DMA gates only ~2 us of PE work.
  * outT is evacuated as bf16 (halves store traffic), un-transposed and
    upcast on the host.

Per-core HBM traffic: 16 MiB in + 4 MiB out; PE streams 5x512 columns per
512-token group.
"""

import os
import numpy as np

P = 128
TPC = 8192          # tokens per core
NCORES = 8
# t-block sizes: big blocks amortize DMA overhead mid-stream, the tapered
# tail keeps the work gated by the last-arriving tile tiny
TBLKS = [2048, 2048, 2048, 1024, 512, 512]
GRP = 512           # tokens per matmul group (PSUM bank = 512 f32)

# DMA arrival order of the 8 d-chunks within each t-block.  Matmul slots
# fire as soon as the later chunk of their pair lands:
#   j6 -> (A@4 || A@6) needs j4,j6 ; j7 -> (B@5 || B@7), (D@7 || D@7)
#   j2 -> (A@0 || A@2)             ; j3 -> (B@1 || B@3) + DVE residual add
LOAD_ORDER = [4, 6, 5, 7, 0, 2, 1, 3]

_STATE = {}


def _bf16():
    import ml_dtypes

    return ml_dtypes.bfloat16


def _pack_w(W_v: np.ndarray) -> np.ndarray:
    """Pack [128, 4, 64] bf16: A, B (shared by all heads), D0, D1 (diag)."""
    W_v = np.asarray(W_v, np.float32)
    w = np.zeros((P, 4, 64), np.float32)
    w[:, 0, :] = W_v.T[0:128]     # A
    w[:, 1, :] = W_v.T[128:256]   # B
    w[0:64, 2, :] = np.eye(64)    # D0: out cols 128:192 += xT rows 896:960
    w[64:128, 3, :] = np.eye(64)  # D1: out cols 192:256 += xT rows 960:1024
    return np.ascontiguousarray(w).astype(_bf16())


def _build_nc(tpc=TPC):
    from contextlib import ExitStack

    import concourse.mybir as mybir
    import concourse.tile as tile
    from concourse import bacc
    from concourse.bass import ds, ts

    bf16 = mybir.dt.bfloat16
    f32 = mybir.dt.float32

    nc = bacc.Bacc("TRN2", target_bir_lowering=False, debug=False)
    xt_h = nc.dram_tensor("xt", [8, P, tpc], bf16, kind="ExternalInput")
    w_h = nc.dram_tensor("w", [P, 4, 64], bf16, kind="ExternalInput")
    o_h = nc.dram_tensor("out", [2, P, tpc], bf16, kind="ExternalOutput")

    ntb = len(TBLKS)
    t0s = [sum(TBLKS[:i]) for i in range(ntb)]
    assert sum(TBLKS) == tpc

    with ExitStack() as ctx:
        tc = ctx.enter_context(tile.TileContext(nc))
        sb = ctx.enter_context(tc.tile_pool(name="sb", bufs=1))
        ps = ctx.enter_context(tc.tile_pool(name="ps", bufs=4, space="PSUM"))

        w_sb = sb.tile([P, 4, 64], bf16)
        nc.sync.dma_start(w_sb[:], w_h[:])
        A, B, D0, D1 = (w_sb[:, k, :] for k in range(4))

        xt_sb = sb.tile([P, 8, tpc], bf16)   # 128 KiB / partition
        out_sb = sb.tile([P, 2, tpc], bf16)  # 32 KiB / partition

        # Enqueue every input load up-front.  The two HWDGE rings stream
        # back-to-back while the PE consumes tiles as they land; the first
        # three tiles ALSO go to the SWDGE (gpsimd) queue, which ramps
        # earlier than the HWDGE rings and is otherwise idle at the start.
        n = 0
        for tb in range(ntb):
            bsl = ds(t0s[tb], TBLKS[tb])
            for j in LOAD_ORDER:
                if n < 3:
                    eng = nc.gpsimd
                else:
                    eng = nc.scalar if n % 2 == 0 else nc.sync
                eng.dma_start(xt_sb[:, j, bsl], xt_h[j, :, bsl])
                n += 1

        def pair(pm, lhs0, j0, lhs1, j1, tsl, start, stop):
            nc.tensor.matmul(pm[0:64, :], lhs0, xt_sb[:, j0, tsl],
                             start=start, stop=stop, tile_position=(0, 0))
            nc.tensor.matmul(pm[64:128, :], lhs1, xt_sb[:, j1, tsl],
                             start=start, stop=stop, tile_position=(0, 64))

        for tb in range(ntb):
            ngrp = TBLKS[tb] // GRP
            tsl = [ds(t0s[tb] + g * GRP, GRP) for g in range(ngrp)]
            pm = {
                (g, cc): ps.tile([P, GRP], f32, tag=f"pm{cc}", name=f"pm{cc}")
                for g in range(ngrp)
                for cc in range(2)
            }
            for g in range(ngrp):  # after j4, j6 land
                pair(pm[(g, 1)], A, 4, A, 6, tsl[g], True, False)
            for g in range(ngrp):  # after j5, j7 land
                pair(pm[(g, 1)], B, 5, B, 7, tsl[g], False, False)
                pair(pm[(g, 1)], D0, 7, D1, 7, tsl[g], False, True)
                # heads 2,3 + residual done -> evacuate via ScalarE
                nc.scalar.copy(out_sb[:, 1, tsl[g]], pm[(g, 1)][:])
            for g in range(ngrp):  # after j0, j2 land
                pair(pm[(g, 0)], A, 0, A, 2, tsl[g], True, False)
            for g in range(ngrp):  # after j1, j3 land
                pair(pm[(g, 0)], B, 1, B, 3, tsl[g], False, True)
                # heads 0,1 + residual (xT chunk 6 is partition-aligned)
                nc.vector.tensor_add(
                    out_sb[:, 0, tsl[g]], pm[(g, 0)][:], xt_sb[:, 6, tsl[g]]
                )
            if tb < ntb - 2:
                # SWDGE so stores don't head-of-line block input HWDGE rings
                bsl = ds(t0s[tb], TBLKS[tb])
                for cc in range(2):
                    nc.gpsimd.dma_start(o_h[cc, :, bsl], out_sb[:, cc, bsl])
            else:
                # input rings are nearly empty by now: ship the tail
                # per-group on the low-latency HWDGE rings
                for g in range(ngrp):
                    for cc in range(2):
                        eng = nc.sync if cc == 0 else nc.scalar
                        eng.dma_start(
                            o_h[cc, :, tsl[g]], out_sb[:, cc, tsl[g]]
                        )

    nc.compile()
    return nc


def _install_ntff_hook():
    """Provide antenv.axon_hooks (absent in this image) so trace=True works."""
    import sys
    import types

    if "antenv.axon_hooks" in sys.modules:
        return
    try:
        import trn_agent_boot.trn_boot as tb

        hook = tb._ntff_profile_via_ctypes("/opt/axon/libaxon_pjrt.so")
    except Exception:
        hook = None
    mod = types.ModuleType("antenv.axon_hooks")
    mod.get_axon_ntff_profile_hook = lambda: hook
    mod.set_axon_ntff_profile_hook = lambda h: None
    sys.modules["antenv.axon_hooks"] = mod
    try:
        import antenv

        antenv.axon_hooks = mod
    except ImportError:
        pass


def kernel(x, W_q=None, W_k=None, W_v=None, **_):
    from concourse.bass_utils import run_bass_kernel_spmd

    if "nc" not in _STATE:
        _STATE["nc"] = _build_nc()
    nc = _STATE["nc"]
    bf16 = _bf16()

    x = np.asarray(x, np.float32)
    b, s, e = x.shape
    xf = x.reshape(b * s, e).astype(bf16)  # one contiguous f32->bf16 pass
    w = _pack_w(W_v)

    in_maps = []
    for c in range(NCORES):
        xtc = np.ascontiguousarray(xf[c * TPC:(c + 1) * TPC].T)  # [1024, TPC]
        in_maps.append({"xt": xtc.reshape(8, P, TPC), "w": w})

    trace = os.environ.get("KERNEL_TRACE", "0") == "1"
    if trace:
        _install_ntff_hook()
    res = run_bass_kernel_spmd(nc, in_maps, core_ids=list(range(NCORES)), trace=trace)
    _STATE["last_results"] = res

    outs = []
    for r in res.results:
        oc = np.asarray(r["out"]).reshape(256, TPC)  # [c, t] bf16
        outs.append(oc.T.astype(np.float32))         # [t, c] f32
    out = np.concatenate(outs, axis=0)
    return out.reshape(b, s, 256)


# revision 9
# speedup vs baseline: 1.1771x; 1.1771x over previous
"""Trainium2 Bass kernel for nn_MultiHeadAttention_45672682226228.

The reference module computes multi-head attention but everything except the
V projection is dead code (DCE'd under jit): the returned value is

    out[b, s, 64*h + q] = x[b, s, 768 + 64*h + q]
                        + sum_d x[b, s, 256*h + d] * W_v[q, d]

i.e. a per-token block-diagonal matmul (4 heads x [256 -> 64]) plus a
residual add of the last head's input slice.  W_q / W_k are unused.

Kernel strategy (v4):
  * Data-parallel over batch B=16 -> 2 batches (8192 tokens) per core.
  * x is pre-transposed and cast to bf16 on the HOST, so the device streams
    xT [1024, 8192] = 8 chunks of [128, 8192] straight into accumulating PE
    matmuls - no on-chip transposes (bf16 error ~3e-3, gate is 2e-2).
  * All 4 heads share W_v, so the only weights are A = W_v.T[0:128] and
    B = W_v.T[128:256], both [128, 64].  M=64 means two matmuls are packed
    side-by-side in the PE array via column tiling (tile_position (0,0) /
    (0,64)), halving PE streaming time:
      outT[  0:128] (heads 0,1): (A@x0 || A@x2), (B@x1 || B@x3)
      outT[128:256] (heads 2,3): (A@x4 || A@x6), (B@x5 || B@x7),
                                 (D0@x7 || D1@x7)   <- residual cols 128:255
    The cc0 residual (xT rows 768:896 = chunk 6, partition-aligned with
    output cols 0:128) is added by the DVE during PSUM evacuation; the cc1
    residual rides two diagonal weight blocks so evacuation can use the
    Scalar engine (which cannot do two-tensor adds).
  * Matmuls are emitted tile-major: each input tile is consumed for all 4
    groups the moment it lands, so the last DMA gates only ~2 us of PE work.
  * outT is evacuated as bf16 (halves store traffic), un-transposed and
    upcast on the host.

Per-core HBM traffic: 16 MiB in + 4 MiB out; PE streams 5x512 columns per
512-token group.
"""

import os
import numpy as np

P = 128
TPC = 8192          # tokens per core
NCORES = 8
# t-block sizes: big blocks amortize DMA overhead mid-stream, the tapered
# tail keeps the work gated by the last-arriving tile tiny
TBLKS = [2048, 2048, 2048, 1024, 512, 512]
GRP = 512           # tokens per matmul group (PSUM bank = 512 f32)

# DMA arrival order of the 8 d-chunks within each t-block.  Matmul slots
# fire as soon as the later chunk of their pair lands:
#   j6 -> (A@4 || A@6) needs j4,j6 ; j7 -> (B@5 || B@7), (D@7 || D@7)
#   j2 -> (A@0 || A@2)             ; j3 -> (B@1 || B@3) + DVE residual add
LOAD_ORDER = [4, 6, 5, 7, 0, 2, 1, 3]

_STATE = {}


def _bf16():
    import ml_dtypes

    return ml_dtypes.bfloat16


def _pack_w(W_v: np.ndarray) -> np.ndarray:
    """Pack [128, 4, 64] bf16: A, B (shared by all heads), D0, D1 (diag)."""
    W_v = np.asarray(W_v, np.float32)
    w = np.zeros((P, 4, 64), np.float32)
    w[:, 0, :] = W_v.T[0:128]     # A
    w[:, 1, :] = W_v.T[128:256]   # B
    w[0:64, 2, :] = np.eye(64)    # D0: out cols 128:192 += xT rows 896:960
    w[64:128, 3, :] = np.eye(64)  # D1: out cols 192:256 += xT rows 960:1024
    return np.ascontiguousarray(w).astype(_bf16())


def _build_nc(tpc=TPC):
    from contextlib import ExitStack

    import concourse.mybir as mybir
    import concourse.tile as tile
    from concourse import bacc
    from concourse.bass import ds, ts

    bf16 = mybir.dt.bfloat16
    f32 = mybir.dt.float32

    nc = bacc.Bacc("TRN2", target_bir_lowering=False, debug=False)
    xt_h = nc.dram_tensor("xt", [8, P, tpc], bf16, kind="ExternalInput")
    w_h = nc.dram_tensor("w", [P, 4, 64], bf16, kind="ExternalInput")
    o_h = nc.dram_tensor("out", [2, P, tpc], bf16, kind="ExternalOutput")

    ntb = len(TBLKS)
    t0s = [sum(TBLKS[:i]) for i in range(ntb)]
    assert sum(TBLKS) == tpc

    with ExitStack() as ctx:
        tc = ctx.enter_context(tile.TileContext(nc))
        sb = ctx.enter_context(tc.tile_pool(name="sb", bufs=1))
        ps = ctx.enter_context(tc.tile_pool(name="ps", bufs=4, space="PSUM"))

        w_sb = sb.tile([P, 4, 64], bf16)
        nc.sync.dma_start(w_sb[:], w_h[:])
        A, B, D0, D1 = (w_sb[:, k, :] for k in range(4))

        xt_sb = sb.tile([P, 8, tpc], bf16)   # 128 KiB / partition
        out_sb = sb.tile([P, 2, tpc], bf16)  # 32 KiB / partition

        # Enqueue every input load up-front; the two HWDGE rings stream them
        # back-to-back while the PE consumes tiles as they land.
        n = 0
        for tb in range(ntb):
            bsl = ds(t0s[tb], TBLKS[tb])
            for j in LOAD_ORDER:
                eng = nc.scalar if n % 2 == 0 else nc.sync
                eng.dma_start(xt_sb[:, j, bsl], xt_h[j, :, bsl])
                n += 1

        def pair(pm, lhs0, j0, lhs1, j1, tsl, start, stop):
            nc.tensor.matmul(pm[0:64, :], lhs0, xt_sb[:, j0, tsl],
                             start=start, stop=stop, tile_position=(0, 0))
            nc.tensor.matmul(pm[64:128, :], lhs1, xt_sb[:, j1, tsl],
                             start=start, stop=stop, tile_position=(0, 64))

        for tb in range(ntb):
            ngrp = TBLKS[tb] // GRP
            tsl = [ds(t0s[tb] + g * GRP, GRP) for g in range(ngrp)]
            pm = {
                (g, cc): ps.tile([P, GRP], f32, tag=f"pm{cc}", name=f"pm{cc}")
                for g in range(ngrp)
                for cc in range(2)
            }
            for g in range(ngrp):  # after j4, j6 land
                pair(pm[(g, 1)], A, 4, A, 6, tsl[g], True, False)
            for g in range(ngrp):  # after j5, j7 land
                pair(pm[(g, 1)], B, 5, B, 7, tsl[g], False, False)
                pair(pm[(g, 1)], D0, 7, D1, 7, tsl[g], False, True)
                # evacuate on DVE (keeps ScalarE a pure DMA dispatcher --
                # no ACT table load, earlier ring start)
                nc.vector.tensor_copy(out_sb[:, 1, tsl[g]], pm[(g, 1)][:])
            for g in range(ngrp):  # after j0, j2 land
                pair(pm[(g, 0)], A, 0, A, 2, tsl[g], True, False)
            for g in range(ngrp):  # after j1, j3 land
                pair(pm[(g, 0)], B, 1, B, 3, tsl[g], False, True)
                # heads 0,1 + residual (xT chunk 6 is partition-aligned)
                nc.vector.tensor_add(
                    out_sb[:, 0, tsl[g]], pm[(g, 0)][:], xt_sb[:, 6, tsl[g]]
                )
            if tb < ntb - 2:
                # SWDGE so stores don't head-of-line block input HWDGE rings
                bsl = ds(t0s[tb], TBLKS[tb])
                for cc in range(2):
                    nc.gpsimd.dma_start(o_h[cc, :, bsl], out_sb[:, cc, bsl])
            else:
                # input rings are nearly empty by now: ship the tail
                # per-group on the low-latency HWDGE rings
                for g in range(ngrp):
                    for cc in range(2):
                        eng = nc.sync if cc == 0 else nc.scalar
                        eng.dma_start(
                            o_h[cc, :, tsl[g]], out_sb[:, cc, tsl[g]]
                        )

    nc.compile()
    return nc


def _install_ntff_hook():
    """Provide antenv.axon_hooks (absent in this image) so trace=True works."""
    import sys
    import types

    if "antenv.axon_hooks" in sys.modules:
        return
    try:
        import trn_agent_boot.trn_boot as tb

        hook = tb._ntff_profile_via_ctypes("/opt/axon/libaxon_pjrt.so")
    except Exception:
        hook = None
    mod = types.ModuleType("antenv.axon_hooks")
    mod.get_axon_ntff_profile_hook = lambda: hook
    mod.set_axon_ntff_profile_hook = lambda h: None
    sys.modules["antenv.axon_hooks"] = mod
    try:
        import antenv

        antenv.axon_hooks = mod
    except ImportError:
        pass


def kernel(x, W_q=None, W_k=None, W_v=None, **_):
    from concourse.bass_utils import run_bass_kernel_spmd

    if "nc" not in _STATE:
        _STATE["nc"] = _build_nc()
    nc = _STATE["nc"]
    bf16 = _bf16()

    x = np.asarray(x, np.float32)
    b, s, e = x.shape
    xf = x.reshape(b * s, e).astype(bf16)  # one contiguous f32->bf16 pass
    w = _pack_w(W_v)

    in_maps = []
    for c in range(NCORES):
        xtc = np.ascontiguousarray(xf[c * TPC:(c + 1) * TPC].T)  # [1024, TPC]
        in_maps.append({"xt": xtc.reshape(8, P, TPC), "w": w})

    trace = os.environ.get("KERNEL_TRACE", "0") == "1"
    if trace:
        _install_ntff_hook()
    res = run_bass_kernel_spmd(nc, in_maps, core_ids=list(range(NCORES)), trace=trace)
    _STATE["last_results"] = res

    outs = []
    for r in res.results:
        oc = np.asarray(r["out"]).reshape(256, TPC)  # [c, t] bf16
        outs.append(oc.T.astype(np.float32))         # [t, c] f32
    out = np.concatenate(outs, axis=0)
    return out.reshape(b, s, 256)
